# revision 2
# baseline (speedup 1.0000x reference)
"""Trainium2 Bass kernel for nn_GATRecommender (8 NeuronCores).

Sharding strategy:
  - Encoders + fusion MLP: data-parallel over the batch (128 rows/core).
  - GAT layer 1 (8 heads): one head per core; node features replicated.
  - GAT layer 2 (1 head): contraction over the 6144 hidden dim sharded by
    head (matmul partials AllReduce'd), then edge/dst-block-parallel
    message passing (3 dst blocks of 128 nodes per core), AllGather.
  - Message passing is expressed as PE matmuls against host-built one-hot
    (edge -> dst) matrices, with exp(e) folded into the one-hot and
    1/denom folded into the PSUM->SBUF epilogue.  Row gathers h[src] use
    the SWDGE dma_gather primitive against DRAM-resident feature tables.

All FLOPs run on device in bf16 with fp32 accumulation.  Host-side work is
restricted to layout (transposes / shards) and integer index preprocessing.
"""
import os
import numpy as np
import ml_dtypes

import concourse.bass as bass
import concourse.bacc as bacc
import concourse.mybir as mybir
import concourse.tile as tile
from concourse import bass_utils

P = 128
NCORES = 8
NU, NB, N, H, HEADS, B = 1024, 2048, 3072, 768, 8, 1024
NIMG = 3
HB = H // P            # 6 channel blocks of 128
NBLK = N // P          # 24 node blocks
BSH = B // NCORES      # 128 batch rows per core
F4 = 4 * H             # 3072 fusion input features
F2 = 2 * H             # 1536
ARW = 896              # AllReduce row width (768 h2 + 2 s2 + pad to 256B-multiple)

BF16 = mybir.dt.bfloat16
F32 = mybir.dt.float32
I16 = mybir.dt.int16
AF = mybir.ActivationFunctionType
ALU = mybir.AluOpType

_nbf = ml_dtypes.bfloat16


def _wrap_idx(idx):
    """[n] -> [128, n/16] int16; index i at (i%16, i//16), replicated to all
    8 gpsimd groups of 16 partitions."""
    idx = np.asarray(idx)
    n = idx.shape[0]
    assert n % 16 == 0
    a = np.zeros((128, n // 16), dtype=np.int16)
    cols = np.arange(n) // 16
    rows = np.arange(n) % 16
    for g in range(8):
        a[rows + 16 * g, cols] = idx.astype(np.int16)
    return a


def _build_blocks(src_s, dst_s, dstblks, nblk_force=None):
    """Edges pre-sorted by dst.  Returns per-dstblk one-hot M [128,nblk,128],
    concatenated padded src/dst index lists, and nblk per dstblk."""
    Ms, srcpad, dstpad, nblks = [], [], [], []
    for d in dstblks:
        sel = (dst_s // P) == d
        sd, dd = src_s[sel], dst_s[sel] - P * d
        n = len(sd)
        nblk = max(1, -(-n // P))
        if nblk_force is not None:
            assert nblk <= nblk_force, (n, nblk_force)
            nblk = nblk_force
        npad = nblk * P
        sp = np.zeros(npad, np.int64)
        sp[:n] = sd
        dp = np.zeros(npad, np.int64)
        dp[:n] = dd + P * d
        M = np.zeros((P, nblk, P), np.float32)
        j = np.arange(n)
        M[j % P, j // P, dd] = 1.0
        Ms.append(M)
        srcpad.append(sp)
        dstpad.append(dp)
        nblks.append(nblk)
    return (np.concatenate(Ms, axis=1),
            np.concatenate(srcpad), np.concatenate(dstpad), nblks)


def host_prep(inputs):
    inp = {k: np.ascontiguousarray(np.asarray(v)) for k, v in inputs.items()}
    user_idx = inp["user_idx"].astype(np.int64)
    business_idx = inp["business_idx"].astype(np.int64)
    ei = inp["edge_index"].astype(np.int64)

    jl = np.full(NB, -1, np.int64)
    jl[business_idx - NU] = np.arange(B)
    bmask = (jl >= 0).astype(np.float32)
    jl = np.where(jl < 0, 0, jl)
    u_mask = np.zeros(NU, np.float32)
    u_mask[user_idx] = 1.0

    src = np.concatenate([ei[0], np.arange(N)])
    dst = np.concatenate([ei[1], np.arange(N)])
    order = np.argsort(dst, kind="stable")
    src_s, dst_s = src[order], dst[order]

    M1, src1, dst1, nblk1 = _build_blocks(src_s, dst_s, range(NBLK))
    T1 = sum(nblk1)

    # layer 2: uniform block count across cores so the SPMD program matches
    nblk2u = 0
    for d in range(NBLK):
        n = int(np.sum((dst_s // P) == d))
        nblk2u = max(nblk2u, -(-n // P))
    l2 = []
    for k in range(NCORES):
        M2, src2, dst2, nblk2 = _build_blocks(
            src_s, dst_s, range(3 * k, 3 * k + 3), nblk_force=nblk2u)
        l2.append(dict(M2=M2, src2=src2, dst2=dst2))
    T2 = 3 * nblk2u

    pr = dict(
        T1=T1, nblk1=nblk1, T2=T2, nblk2u=nblk2u,
        M1=M1.astype(_nbf),
        src1w=_wrap_idx(src1), dst1w=_wrap_idx(dst1),
        jlw=_wrap_idx(jl),
        u_mask_b=np.broadcast_to(u_mask.astype(_nbf), (P, NU)).copy(),
        bm025_b=np.broadcast_to((0.25 * bmask).astype(_nbf), (P, NB)).copy(),
        ident=np.eye(P, dtype=_nbf),
        l2=[dict(M2=c["M2"].astype(_nbf), src2w=_wrap_idx(c["src2"]),
                 dst2w=_wrap_idx(c["dst2"])) for c in l2],
        uiw=[_wrap_idx(user_idx[k * BSH:(k + 1) * BSH]) for k in range(NCORES)],
        biw=[_wrap_idx(business_idx[k * BSH:(k + 1) * BSH]) for k in range(NCORES)],
        has_b1=bool(np.any(inp["b1"] != 0)),
        has_b2=bool(np.any(inp["b2"] != 0)),
        bf3_val=float(inp["bf3"][0]),
        inp=inp,
    )
    return pr


def build_program(pr, debug=False):
    T1, nblk1, T2, nblk2u = pr["T1"], pr["nblk1"], pr["T2"], pr["nblk2u"]
    has_b1, has_b2 = pr["has_b1"], pr["has_b2"]

    nc = bacc.Bacc("TRN2", target_bir_lowering=False, debug=False,
                   num_devices=NCORES, num_swdge_queues=2)
    D = nc.dram_tensor

    # ---- inputs ----
    t_text = D("text_clsT", [H, BSH], F32, kind="ExternalInput")
    t_img = D("imgT", [NIMG, H, BSH], F32, kind="ExternalInput")
    t_bizf = D("bizfT", [3, BSH], F32, kind="ExternalInput")
    t_wtext = D("W_text", [H, H], F32, kind="ExternalInput")
    t_wimg = D("W_img", [H, H], F32, kind="ExternalInput")
    t_wbf = D("W_bf", [3, H], F32, kind="ExternalInput")
    t_btext = D("b_text", [H], F32, kind="ExternalInput")
    t_bimg = D("b_img", [H], F32, kind="ExternalInput")
    t_bbf = D("b_bf", [H], F32, kind="ExternalInput")
    t_usertT = D("user_tableT", [H, NU], F32, kind="ExternalInput")
    t_biztT = D("biz_tableT", [H, NB], F32, kind="ExternalInput")
    t_w1 = D("W1k", [H, H], F32, kind="ExternalInput")
    t_w2 = D("W2k", [H, H], F32, kind="ExternalInput")
    t_a1 = D("a1k", [H, 2], F32, kind="ExternalInput")
    t_w1T = D("W1kT", [H, H], F32, kind="ExternalInput")
    t_w2T = D("W2kT", [H, H], F32, kind="ExternalInput")
    t_a2 = D("a2", [H, 2], F32, kind="ExternalInput")
    t_wf1 = D("Wf1", [F4, F2], F32, kind="ExternalInput")
    t_wf2 = D("Wf2", [F2, H], F32, kind="ExternalInput")
    t_wf3 = D("Wf3", [H, 1], F32, kind="ExternalInput")
    t_bf1 = D("bf1", [F2], F32, kind="ExternalInput")
    t_bf2 = D("bf2", [H], F32, kind="ExternalInput")
    t_m1 = D("M1", [P, T1, P], BF16, kind="ExternalInput")
    t_s1w = D("src1w", [P, T1 * 8], I16, kind="ExternalInput")
    t_d1w = D("dst1w", [P, T1 * 8], I16, kind="ExternalInput")
    t_m2 = D("M2", [P, T2, P], BF16, kind="ExternalInput")
    t_s2w = D("src2w", [P, T2 * 8], I16, kind="ExternalInput")
    t_d2w = D("dst2w", [P, T2 * 8], I16, kind="ExternalInput")
    t_jlw = D("jlw", [P, NB // 16], I16, kind="ExternalInput")
    t_uiw = D("uiw", [P, BSH // 16], I16, kind="ExternalInput")
    t_biw = D("biw", [P, BSH // 16], I16, kind="ExternalInput")
    t_um = D("u_mask_b", [P, NU], BF16, kind="ExternalInput")
    t_bm = D("bm025_b", [P, NB], BF16, kind="ExternalInput")
    t_id = D("ident", [P, P], BF16, kind="ExternalInput")
    if has_b1:
        t_b1b = D("b1_b", [P, H], F32, kind="ExternalInput")
    if has_b2:
        t_b2b = D("b2_b", [P, H], F32, kind="ExternalInput")

    t_y = D("y", [P, 1], F32, kind="ExternalOutput")
    dbg = {}
    if debug:
        dbg["x2"] = D("dbg_x2", [P, NBLK, H], F32, kind="ExternalOutput")
        dbg["den"] = D("dbg_den", [P, NBLK], F32, kind="ExternalOutput")
        dbg["xo"] = D("dbg_xo", [N, H], BF16, kind="ExternalOutput")
        dbg["svec"] = D("dbg_svec", [P, NBLK, 2], F32, kind="ExternalOutput")

    rg = [list(range(NCORES))]

    with tile.TileContext(nc) as tc:
        sy = nc.sync
        gp = nc.gpsimd
        ve = nc.vector
        sc = nc.scalar
        te = nc.tensor

        with (tc.tile_pool(name="pp", bufs=1) as pp,
              tc.tile_pool(name="ps_big", bufs=2, space="PSUM") as ps_big,
              tc.tile_pool(name="ps_mid", bufs=2, space="PSUM") as ps_mid,
              tc.tile_pool(name="ps_sml", bufs=2, space="PSUM") as ps_sml,
              tc.tile_pool(name="dram", bufs=1, space="DRAM") as dram):

            # cross-phase persistent tiles
            textT = pp.tile([P, HB, BSH], BF16, tag="textT")
            imgT = pp.tile([P, HB, BSH], BF16, tag="imgT")
            s_ag_in = dram.tile([BSH, H], BF16)
            s_full = dram.tile([B, H], BF16)
            fat_dram = dram.tile([N, 64], F32)
            h_dram = dram.tile([N, H], BF16)
            x2_dram = dram.tile([N, H], BF16)
            ar_in = dram.tile([N, ARW], BF16)
            ar_out = dram.tile([N, ARW], BF16)
            fat2_dram = dram.tile([N, 64], F32)
            ag_in = dram.tile([3 * P, H], BF16)
            xo_dram = dram.tile([N, H], BF16)

            # ====== phase 0: encoders (transposed, batch shard) ======
            with (tc.tile_pool(name="ep", bufs=1) as ep,
                  tc.tile_pool(name="ep2", bufs=2) as ep2):
                wtext = ep.tile([P, HB, H], BF16, tag="wtext")
                gp.dma_start(wtext[:], t_wtext[:].rearrange("(a p) c -> p a c", p=P))
                wimg = ep.tile([P, HB, H], BF16, tag="wimg")
                gp.dma_start(wimg[:], t_wimg[:].rearrange("(a p) c -> p a c", p=P))
                wbf = ep.tile([3, H], BF16, tag="wbf")
                gp.dma_start(wbf[:], t_wbf[:])
                btext = ep.tile([P, HB], F32, tag="btext")
                sy.dma_start(btext[:], t_btext[:].rearrange("(a p) -> p a", p=P))
                bimg = ep.tile([P, HB], F32, tag="bimg")
                sy.dma_start(bimg[:], t_bimg[:].rearrange("(a p) -> p a", p=P))
                bbf = ep.tile([P, HB], F32, tag="bbf")
                sy.dma_start(bbf[:], t_bbf[:].rearrange("(a p) -> p a", p=P))

                tct = ep.tile([P, HB, BSH], BF16, tag="tct")
                gp.dma_start(tct[:], t_text[:].rearrange("(a p) b -> p a b", p=P))
                img0 = ep2.tile([P, HB, BSH], BF16, tag="imgl")
                gp.dma_start(img0[:], t_img[0].rearrange("(a p) b -> p a b", p=P))
                img1 = ep2.tile([P, HB, BSH], BF16, tag="imgl")
                gp.dma_start(img1[:], t_img[1].rearrange("(a p) b -> p a b", p=P))
                img2 = ep.tile([P, HB, BSH], BF16, tag="imgl3")
                gp.dma_start(img2[:], t_img[2].rearrange("(a p) b -> p a b", p=P))
                imgsum = ep.tile([P, HB, BSH], BF16, tag="imgsum")
                ve.tensor_tensor(imgsum[:], img0[:], img1[:], op=ALU.add)
                ve.tensor_tensor(imgsum[:], imgsum[:], img2[:], op=ALU.add)
                bizf = ep.tile([3, BSH], BF16, tag="bizf")
                gp.dma_start(bizf[:], t_bizf[:])

                sT = ep.tile([P, HB, BSH], BF16, tag="sT")
                for co in range(HB):
                    pt = ps_sml.tile([P, BSH], F32, tag="enc")
                    for ci in range(HB):
                        te.matmul(pt[:], wtext[:, ci, co * P:(co + 1) * P],
                                  tct[:, ci, :], start=(ci == 0),
                                  stop=(ci == HB - 1))
                    ve.tensor_scalar(textT[:, co, :], pt[:], btext[:, co:co + 1],
                                     None, ALU.add)
                    pt2 = ps_sml.tile([P, BSH], F32, tag="enc")
                    for ci in range(HB):
                        te.matmul(pt2[:], wimg[:, ci, co * P:(co + 1) * P],
                                  imgsum[:, ci, :], start=(ci == 0),
                                  stop=(ci == HB - 1))
                    ve.tensor_scalar(imgT[:, co, :], pt2[:], 1.0 / 3.0,
                                     bimg[:, co:co + 1], ALU.mult, ALU.add)
                    pt3 = ps_sml.tile([P, BSH], F32, tag="enc")
                    te.matmul(pt3[:], wbf[:, co * P:(co + 1) * P], bizf[:],
                              start=True, stop=True)
                    ve.tensor_scalar(sT[:, co, :], pt3[:], bbf[:, co:co + 1],
                                     None, ALU.add)
                    ve.tensor_tensor(sT[:, co, :], sT[:, co, :], textT[:, co, :],
                                     op=ALU.add)
                    ve.tensor_tensor(sT[:, co, :], sT[:, co, :], imgT[:, co, :],
                                     op=ALU.add)

                # s row-major shard -> DRAM -> AllGather
                ident = ep.tile([P, P], BF16, tag="ident")
                sy.dma_start(ident[:], t_id[:])
                srow = ep.tile([P, H], BF16, tag="srow")
                for ci in range(HB):
                    ptt = ps_sml.tile([P, P], BF16, tag="enc")
                    te.transpose(ptt[:], sT[:, ci, :], ident[:])
                    ve.tensor_copy(srow[:, ci * P:(ci + 1) * P], ptt[:])
                gp.dma_start(s_ag_in[:], srow[:])
            gp.collective_compute("AllGather", ALU.bypass, replica_groups=rg,
                                  ins=[s_ag_in.opt()], outs=[s_full.opt()])

            # ====== phase 1: build x^T + layer-1 local matmuls ======
            xT = pp.tile([P, HB, N], BF16, tag="xT")
            with tc.tile_pool(name="xb", bufs=1) as xp:
                jlidx = xp.tile([P, NB // 16], I16, tag="jlidx")
                sy.dma_start(jlidx[:], t_jlw[:])
                umask = xp.tile([P, NU], BF16, tag="umask")
                sy.dma_start(umask[:], t_um[:])
                bmask = xp.tile([P, NB], BF16, tag="bmask")
                sy.dma_start(bmask[:], t_bm[:])

                ut = xp.tile([P, HB, NU], BF16, tag="ut")
                gp.dma_start(ut[:], t_usertT[:].rearrange("(a p) n -> p a n", p=P))
                for c in range(HB):
                    ve.tensor_tensor(xT[:, c, 0:NU], ut[:, c, :], umask[:],
                                     op=ALU.mult)
                sg = xp.tile([P, HB, NB], BF16, tag="sgath")
                gp.dma_gather(sg[:], s_full[:], jlidx[:], num_idxs=NB,
                              num_idxs_reg=NB, elem_size=H, transpose=True, single_packet=False)
                bt = xp.tile([P, HB, NB], BF16, tag="bt")
                gp.dma_start(bt[:], t_biztT[:].rearrange("(a p) n -> p a n", p=P))
                for c in range(HB):
                    ve.tensor_tensor(sg[:, c, :], sg[:, c, :], bt[:, c, :],
                                     op=ALU.add)
                    ve.tensor_tensor(xT[:, c, NU:N], sg[:, c, :], bmask[:],
                                     op=ALU.mult)

            # ====== layer 1 ======
            with (tc.tile_pool(name="l1", bufs=1) as l1p,
                  tc.tile_pool(name="l1d", bufs=2) as l1d,
                  tc.tile_pool(name="l1t", bufs=3) as l1t):
                w1 = l1p.tile([P, HB, H], BF16, tag="w1")
                gp.dma_start(w1[:], t_w1[:].rearrange("(a p) c -> p a c", p=P))
                a1 = l1p.tile([P, HB, 2], BF16, tag="a1")
                gp.dma_start(a1[:], t_a1[:].rearrange("(a p) c -> p a c", p=P))
                w1T = l1p.tile([P, HB, H], BF16, tag="w1T")
                gp.dma_start(w1T[:], t_w1T[:].rearrange("(a p) c -> p a c", p=P))
                ws1 = l1p.tile([P, HB, 2], BF16, tag="ws1")
                for f in range(HB):
                    pw = ps_sml.tile([P, 2], F32, tag="vec")
                    for co in range(HB):
                        te.matmul(pw[:], w1T[:, co, f * P:(f + 1) * P],
                                  a1[:, co, :], start=(co == 0),
                                  stop=(co == HB - 1))
                    ve.tensor_copy(ws1[:, f, :], pw[:])

                svec = l1p.tile([P, NBLK, 2], F32, tag="svec")
                for nb in range(NBLK):
                    pv = ps_sml.tile([P, 2], F32, tag="vec")
                    for ci in range(HB):
                        te.matmul(pv[:], xT[:, ci, nb * P:(nb + 1) * P],
                                  ws1[:, ci, :], start=(ci == 0),
                                  stop=(ci == HB - 1))
                    ve.tensor_copy(svec[:, nb, :], pv[:])
                if debug:
                    sy.dma_start(dbg["svec"][:], svec[:])

                fat_sb = l1p.tile([P, NBLK, 64], F32, tag="fat_sb")
                ve.memset(fat_sb[:], 0.0)
                ve.tensor_copy(fat_sb[:, :, 0:2], svec[:])
                gp.dma_start(fat_dram[:].rearrange("(a p) c -> p a c", p=P),
                             fat_sb[:])

                # h = x @ W1_k  (row-major), streamed to DRAM
                for nb in range(NBLK):
                    ph1 = ps_big.tile([P, 512], F32, tag="big")
                    ph2 = ps_mid.tile([P, 256], F32, tag="mid")
                    for ci in range(HB):
                        te.matmul(ph1[:], xT[:, ci, nb * P:(nb + 1) * P],
                                  w1[:, ci, 0:512], start=(ci == 0),
                                  stop=(ci == HB - 1))
                    for ci in range(HB):
                        te.matmul(ph2[:], xT[:, ci, nb * P:(nb + 1) * P],
                                  w1[:, ci, 512:H], start=(ci == 0),
                                  stop=(ci == HB - 1))
                    hst = l1t.tile([P, H], BF16, tag="hst")
                    ve.tensor_copy(hst[:, 0:512], ph1[:])
                    ve.tensor_copy(hst[:, 512:H], ph2[:])
                    sy.dma_start(h_dram[nb * P:(nb + 1) * P, :], hst[:])

                # --- edge phase ---
                s1idx = l1p.tile([P, T1 * 8], I16, tag="s1idx")
                sy.dma_start(s1idx[:], t_s1w[:])
                d1idx = l1p.tile([P, T1 * 8], I16, tag="d1idx")
                sy.dma_start(d1idx[:], t_d1w[:])

                ee = l1p.tile([P, T1], F32, tag="ee")
                eebf = l1p.tile([P, T1], BF16, tag="eebf")
                off1 = np.concatenate([[0], np.cumsum(nblk1)]).astype(int)
                groups = [(0, 6), (6, 12), (12, 18), (18, 24)]
                for g0, g1 in groups:
                    o0, o1 = int(off1[g0]), int(off1[g1])
                    cnt = o1 - o0
                    gs = l1d.tile([P, cnt, 64], F32, tag="fatg")
                    gp.dma_gather(gs[:], fat_dram[:], s1idx[:, o0 * 8:o1 * 8],
                                  num_idxs=cnt * P, num_idxs_reg=cnt * P,
                                  elem_size=64, single_packet=False)
                    gd = l1d.tile([P, cnt, 64], F32, tag="fatg2")
                    gp.dma_gather(gd[:], fat_dram[:], d1idx[:, o0 * 8:o1 * 8],
                                  num_idxs=cnt * P, num_idxs_reg=cnt * P,
                                  elem_size=64, single_packet=False)
                    # e = s_src[src] + s_dst[dst]
                    ve.tensor_tensor(ee[:, o0:o1], gs[:, :, 0], gd[:, :, 1],
                                     op=ALU.add)
                et = l1p.tile([P, T1], F32, tag="et")
                ve.tensor_scalar(et[:], ee[:], 0.2, None, ALU.mult)
                ve.tensor_tensor(ee[:], ee[:], et[:], op=ALU.max)
                sc.activation(ee[:], ee[:], AF.Exp)
                ve.tensor_copy(eebf[:], ee[:])

                den = l1p.tile([P, NBLK], F32, tag="den")
                recip = l1p.tile([P, NBLK], F32, tag="recip")
                if has_b1:
                    b1b = l1p.tile([P, H], F32, tag="b1b")
                    sy.dma_start(b1b[:], t_b1b[:])

                for d in range(NBLK):
                    nblk = nblk1[d]
                    o = int(off1[d])
                    m1 = l1t.tile([P, nblk, P], BF16, tag="m1")
                    sy.dma_start(m1[:], t_m1[:, o:o + nblk, :])
                    pa = ps_sml.tile([P, 2], F32, tag="vec")
                    for b in range(nblk):
                        te.matmul(pa[:, 0:1], m1[:, b, :],
                                  eebf[:, o + b:o + b + 1],
                                  start=(b == 0), stop=(b == nblk - 1))
                    ve.tensor_scalar(den[:, d:d + 1], pa[:, 0:1], 1e-16, None,
                                     ALU.add)
                    ve.reciprocal(recip[:, d:d + 1], den[:, d:d + 1])
                    mbe = l1d.tile([P, nblk, P], BF16, tag="mbe")
                    for b in range(nblk):
                        ve.tensor_scalar(mbe[:, b, :], m1[:, b, :],
                                         ee[:, o + b:o + b + 1], None, ALU.mult)
                    gh = l1d.tile([P, nblk, H], BF16, tag="gh")
                    gp.dma_gather(gh[:], h_dram[:],
                                  s1idx[:, o * 8:(o + nblk) * 8],
                                  num_idxs=nblk * P, num_idxs_reg=nblk * P,
                                  elem_size=H, single_packet=False)
                    pb1 = ps_big.tile([P, 512], F32, tag="big")
                    pb2 = ps_mid.tile([P, 256], F32, tag="mid")
                    for b in range(nblk):
                        te.matmul(pb1[:], mbe[:, b, :], gh[:, b, 0:512],
                                  start=(b == 0), stop=(b == nblk - 1))
                    for b in range(nblk):
                        te.matmul(pb2[:], mbe[:, b, :], gh[:, b, 512:H],
                                  start=(b == 0), stop=(b == nblk - 1))
                    x2st = l1t.tile([P, H], BF16, tag="hst")
                    if has_b1:
                        tmp = l1t.tile([P, H], F32, tag="tmpb")
                        ve.tensor_scalar(tmp[:, 0:512], pb1[:],
                                         recip[:, d:d + 1], None, ALU.mult)
                        ve.tensor_scalar(tmp[:, 512:H], pb2[:],
                                         recip[:, d:d + 1], None, ALU.mult)
                        ve.tensor_tensor(tmp[:], tmp[:], b1b[:], op=ALU.add)
                        ve.tensor_scalar(x2st[:], tmp[:], 0.0, None, ALU.max)
                    else:
                        ve.tensor_scalar(x2st[:, 0:512], pb1[:],
                                         recip[:, d:d + 1], 0.0, ALU.mult,
                                         ALU.max)
                        ve.tensor_scalar(x2st[:, 512:H], pb2[:],
                                         recip[:, d:d + 1], 0.0, ALU.mult,
                                         ALU.max)
                    sy.dma_start(x2_dram[d * P:(d + 1) * P, :], x2st[:])
                    if debug:
                        dx = l1t.tile([P, H], F32, tag="dbgx")
                        ve.tensor_copy(dx[:], x2st[:])
                        sy.dma_start(dbg["x2"][:, d, :], dx[:])
                if debug:
                    sy.dma_start(dbg["den"][:], den[:])

            # ====== layer 2 ======
            x2T = pp.tile([P, HB, N], BF16, tag="xT")
            with (tc.tile_pool(name="l2", bufs=1) as l2p,
                  tc.tile_pool(name="l2d", bufs=2) as l2d,
                  tc.tile_pool(name="l2t", bufs=3) as l2t):
                for c in range(HB):
                    sy.dma_start_transpose(x2T[:, c, :],
                                           x2_dram[:, c * P:(c + 1) * P])
                w2 = l2p.tile([P, HB, H], BF16, tag="w2")
                gp.dma_start(w2[:], t_w2[:].rearrange("(a p) c -> p a c", p=P))
                a2 = l2p.tile([P, HB, 2], BF16, tag="a2")
                gp.dma_start(a2[:], t_a2[:].rearrange("(a p) c -> p a c", p=P))
                w2T = l2p.tile([P, HB, H], BF16, tag="w2T")
                gp.dma_start(w2T[:], t_w2T[:].rearrange("(a p) c -> p a c", p=P))
                ws2 = l2p.tile([P, HB, 2], BF16, tag="ws2")
                for f in range(HB):
                    pw = ps_sml.tile([P, 2], F32, tag="vec")
                    for co in range(HB):
                        te.matmul(pw[:], w2T[:, co, f * P:(f + 1) * P],
                                  a2[:, co, :], start=(co == 0),
                                  stop=(co == HB - 1))
                    ve.tensor_copy(ws2[:, f, :], pw[:])

                for nb in range(NBLK):
                    ph1 = ps_big.tile([P, 512], F32, tag="big")
                    ph2 = ps_mid.tile([P, 256], F32, tag="mid")
                    pv = ps_sml.tile([P, 2], F32, tag="vec")
                    for ci in range(HB):
                        te.matmul(ph1[:], x2T[:, ci, nb * P:(nb + 1) * P],
                                  w2[:, ci, 0:512], start=(ci == 0),
                                  stop=(ci == HB - 1))
                    for ci in range(HB):
                        te.matmul(ph2[:], x2T[:, ci, nb * P:(nb + 1) * P],
                                  w2[:, ci, 512:H], start=(ci == 0),
                                  stop=(ci == HB - 1))
                    for ci in range(HB):
                        te.matmul(pv[:], x2T[:, ci, nb * P:(nb + 1) * P],
                                  ws2[:, ci, :], start=(ci == 0),
                                  stop=(ci == HB - 1))
                    ast = l2t.tile([P, ARW], BF16, tag="ast")
                    ve.memset(ast[:, 770:ARW], 0.0)
                    ve.tensor_copy(ast[:, 0:512], ph1[:])
                    ve.tensor_copy(ast[:, 512:H], ph2[:])
                    ve.tensor_copy(ast[:, H:770], pv[:])
                    sy.dma_start(ar_in[nb * P:(nb + 1) * P, :], ast[:])
                gp.collective_compute("AllReduce", ALU.add, replica_groups=rg,
                                      ins=[ar_in.opt()], outs=[ar_out.opt()])

                # --- layer 2 edge phase (3 local dstblks) ---
                svec2 = l2p.tile([P, NBLK, 2], F32, tag="svec2")
                s2bf = l2d.tile([P, NBLK, 2], BF16, tag="s2bf")
                sy.dma_start(s2bf[:],
                             ar_out[:, H:770].rearrange("(a p) c -> p a c", p=P))
                ve.tensor_copy(svec2[:], s2bf[:])
                fat2_sb = l2p.tile([P, NBLK, 64], F32, tag="fat_sb")
                ve.memset(fat2_sb[:], 0.0)
                ve.tensor_copy(fat2_sb[:, :, 0:2], svec2[:])
                gp.dma_start(fat2_dram[:].rearrange("(a p) c -> p a c", p=P),
                             fat2_sb[:])

                s2idx = l2p.tile([P, T2 * 8], I16, tag="s2idx")
                sy.dma_start(s2idx[:], t_s2w[:])
                d2idx = l2p.tile([P, T2 * 8], I16, tag="d2idx")
                sy.dma_start(d2idx[:], t_d2w[:])

                ee2 = l2p.tile([P, T2], F32, tag="ee2")
                gs2 = l2d.tile([P, T2, 64], F32, tag="fatg")
                gp.dma_gather(gs2[:], fat2_dram[:], s2idx[:], num_idxs=T2 * P,
                              num_idxs_reg=T2 * P, elem_size=64, single_packet=False)
                gd2 = l2d.tile([P, T2, 64], F32, tag="fatg2")
                gp.dma_gather(gd2[:], fat2_dram[:], d2idx[:], num_idxs=T2 * P,
                              num_idxs_reg=T2 * P, elem_size=64, single_packet=False)
                ve.tensor_tensor(ee2[:], gs2[:, :, 0], gd2[:, :, 1], op=ALU.add)
                et2 = l2p.tile([P, T2], F32, tag="et2")
                ve.tensor_scalar(et2[:], ee2[:], 0.2, None, ALU.mult)
                ve.tensor_tensor(ee2[:], ee2[:], et2[:], op=ALU.max)
                sc.activation(ee2[:], ee2[:], AF.Exp)
                ee2bf = l2p.tile([P, T2], BF16, tag="ee2bf")
                ve.tensor_copy(ee2bf[:], ee2[:])

                den2 = l2p.tile([P, 3], F32, tag="den2")
                recip2 = l2p.tile([P, 3], F32, tag="recip2")
                if has_b2:
                    b2b = l2p.tile([P, H], F32, tag="b2b")
                    sy.dma_start(b2b[:], t_b2b[:])

                for dl in range(3):
                    o = dl * nblk2u
                    m2 = l2t.tile([P, nblk2u, P], BF16, tag="m1")
                    sy.dma_start(m2[:], t_m2[:, o:o + nblk2u, :])
                    pa = ps_sml.tile([P, 2], F32, tag="vec")
                    for b in range(nblk2u):
                        te.matmul(pa[:, 0:1], m2[:, b, :],
                                  ee2bf[:, o + b:o + b + 1],
                                  start=(b == 0), stop=(b == nblk2u - 1))
                    ve.tensor_scalar(den2[:, dl:dl + 1], pa[:, 0:1], 1e-16,
                                     None, ALU.add)
                    ve.reciprocal(recip2[:, dl:dl + 1], den2[:, dl:dl + 1])
                    mbe = l2d.tile([P, nblk2u, P], BF16, tag="mbe")
                    for b in range(nblk2u):
                        ve.tensor_scalar(mbe[:, b, :], m2[:, b, :],
                                         ee2[:, o + b:o + b + 1], None,
                                         ALU.mult)
                    gh = l2d.tile([P, nblk2u, H], BF16, tag="gh")
                    gp.dma_gather(gh[:], ar_out[:, 0:H],
                                  s2idx[:, o * 8:(o + nblk2u) * 8],
                                  num_idxs=nblk2u * P, num_idxs_reg=nblk2u * P,
                                  elem_size=H, elem_step=ARW, single_packet=False)
                    pb1 = ps_big.tile([P, 512], F32, tag="big")
                    pb2 = ps_mid.tile([P, 256], F32, tag="mid")
                    for b in range(nblk2u):
                        te.matmul(pb1[:], mbe[:, b, :], gh[:, b, 0:512],
                                  start=(b == 0), stop=(b == nblk2u - 1))
                    for b in range(nblk2u):
                        te.matmul(pb2[:], mbe[:, b, :], gh[:, b, 512:H],
                                  start=(b == 0), stop=(b == nblk2u - 1))
                    xost = l2t.tile([P, H], BF16, tag="hst")
                    if has_b2:
                        tmp = l2t.tile([P, H], F32, tag="tmpb")
                        ve.tensor_scalar(tmp[:, 0:512], pb1[:],
                                         recip2[:, dl:dl + 1], None, ALU.mult)
                        ve.tensor_scalar(tmp[:, 512:H], pb2[:],
                                         recip2[:, dl:dl + 1], None, ALU.mult)
                        ve.tensor_tensor(xost[:], tmp[:], b2b[:], op=ALU.add)
                    else:
                        ve.tensor_scalar(xost[:, 0:512], pb1[:],
                                         recip2[:, dl:dl + 1], None, ALU.mult)
                        ve.tensor_scalar(xost[:, 512:H], pb2[:],
                                         recip2[:, dl:dl + 1], None, ALU.mult)
                    sy.dma_start(ag_in[dl * P:(dl + 1) * P, :], xost[:])
                gp.collective_compute("AllGather", ALU.bypass, replica_groups=rg,
                                      ins=[ag_in.opt()], outs=[xo_dram.opt()])
                if debug:
                    xodbg = l2d.tile([P, NBLK, H], BF16, tag="xodbg")
                    gp.dma_start(xodbg[:],
                                 xo_dram[:].rearrange("(a p) c -> p a c", p=P))
                    gp.dma_start(dbg["xo"][:].rearrange("(a p) c -> p a c", p=P),
                                 xodbg[:])

            # ====== fusion MLP (batch shard) ======
            with (tc.tile_pool(name="fu", bufs=1) as fp,
                  tc.tile_pool(name="fud", bufs=2) as fd):
                uidx = fp.tile([P, BSH // 16], I16, tag="uidx")
                sy.dma_start(uidx[:], t_uiw[:])
                bidx = fp.tile([P, BSH // 16], I16, tag="bidx")
                sy.dma_start(bidx[:], t_biw[:])
                xuT = fp.tile([P, HB, BSH], BF16, tag="xuT")
                gp.dma_gather(xuT[:], xo_dram[:], uidx[:], num_idxs=BSH,
                              num_idxs_reg=BSH, elem_size=H, transpose=True, single_packet=False)
                xbT = fp.tile([P, HB, BSH], BF16, tag="xbT")
                gp.dma_gather(xbT[:], xo_dram[:], bidx[:], num_idxs=BSH,
                              num_idxs_reg=BSH, elem_size=H, transpose=True, single_packet=False)

                bf1 = fp.tile([P, F2 // P], F32, tag="bf1")
                sy.dma_start(bf1[:], t_bf1[:].rearrange("(a p) -> p a", p=P))
                bf2 = fp.tile([P, HB], F32, tag="bf2")
                sy.dma_start(bf2[:], t_bf2[:].rearrange("(a p) -> p a", p=P))

                cat_tiles = [xuT, xbT, textT, imgT]
                h1fT = fp.tile([P, F2 // P, BSH], BF16, tag="h1fT")
                for ob in range(F2 // P):
                    wf1 = fd.tile([P, F4 // P, P], BF16, tag="wf1")
                    gp.dma_start(
                        wf1[:],
                        t_wf1[:, ob * P:(ob + 1) * P].rearrange(
                            "(a p) c -> p a c", p=P))
                    pf = ps_sml.tile([P, BSH], F32, tag="enc")
                    for fb in range(F4 // P):
                        rhs = cat_tiles[fb // HB][:, fb % HB, :]
                        te.matmul(pf[:], wf1[:, fb, :], rhs, start=(fb == 0),
                                  stop=(fb == F4 // P - 1))
                    ve.tensor_scalar(h1fT[:, ob, :], pf[:], bf1[:, ob:ob + 1],
                                     0.0, ALU.add, ALU.max)

                h2fT = fp.tile([P, HB, BSH], BF16, tag="h2fT")
                for ob in range(HB):
                    wf2 = fd.tile([P, F2 // P, P], BF16, tag="wf2")
                    gp.dma_start(
                        wf2[:],
                        t_wf2[:, ob * P:(ob + 1) * P].rearrange(
                            "(a p) c -> p a c", p=P))
                    pf = ps_sml.tile([P, BSH], F32, tag="enc")
                    for fb in range(F2 // P):
                        te.matmul(pf[:], wf2[:, fb, :], h1fT[:, fb, :],
                                  start=(fb == 0), stop=(fb == F2 // P - 1))
                    ve.tensor_scalar(h2fT[:, ob, :], pf[:], bf2[:, ob:ob + 1],
                                     0.0, ALU.add, ALU.max)

                wf3 = fp.tile([P, HB, 1], BF16, tag="wf3")
                gp.dma_start(wf3[:], t_wf3[:].rearrange("(a p) c -> p a c", p=P))
                py = ps_sml.tile([P, 2], F32, tag="vec")
                for c in range(HB):
                    te.matmul(py[:, 0:1], h2fT[:, c, :], wf3[:, c, :],
                              start=(c == 0), stop=(c == HB - 1))
                ysb = fp.tile([P, 1], F32, tag="ysb")
                ve.tensor_scalar(ysb[:], py[:, 0:1], pr["bf3_val"], None,
                                 ALU.add)
                sy.dma_start(t_y[:], ysb[:])

    nc.compile()
    return nc


def make_in_maps(pr):
    inp = pr["inp"]
    f32 = np.float32
    text_clsT = np.ascontiguousarray(inp["text_cls"].T.astype(f32))
    imgT = np.ascontiguousarray(inp["img_cls"].transpose(1, 2, 0).astype(f32))
    bizfT = np.ascontiguousarray(inp["biz_feats"].T.astype(f32))
    usertT = np.ascontiguousarray(inp["user_table"].T.astype(f32))
    biztT = np.ascontiguousarray(inp["biz_table"].T.astype(f32))
    a2 = np.ascontiguousarray(
        np.stack([inp["att_src2"][0], inp["att_dst2"][0]], axis=1).astype(f32))
    in_maps = []
    for k in range(NCORES):
        sl = slice(k * BSH, (k + 1) * BSH)
        m = dict(
            text_clsT=text_clsT[:, sl].copy(),
            imgT=imgT[:, :, sl].copy(),
            bizfT=bizfT[:, sl].copy(),
            W_text=inp["W_text"].astype(f32),
            W_img=inp["W_img"].astype(f32),
            W_bf=inp["W_bf"].astype(f32),
            b_text=inp["b_text"].astype(f32),
            b_img=inp["b_img"].astype(f32),
            b_bf=inp["b_bf"].astype(f32),
            user_tableT=usertT,
            biz_tableT=biztT,
            W1k=np.ascontiguousarray(inp["W1"][:, k * H:(k + 1) * H].astype(f32)),
            W1kT=np.ascontiguousarray(inp["W1"][:, k * H:(k + 1) * H].T.astype(f32)),
            W2kT=np.ascontiguousarray(inp["W2"][k * H:(k + 1) * H, :].T.astype(f32)),
            W2k=np.ascontiguousarray(inp["W2"][k * H:(k + 1) * H, :].astype(f32)),
            a1k=np.ascontiguousarray(
                np.stack([inp["att_src1"][k], inp["att_dst1"][k]],
                         axis=1).astype(f32)),
            a2=a2,
            Wf1=inp["Wf1"].astype(f32),
            Wf2=inp["Wf2"].astype(f32),
            Wf3=inp["Wf3"].astype(f32),
            bf1=inp["bf1"].astype(f32),
            bf2=inp["bf2"].astype(f32),
            M1=pr["M1"],
            src1w=pr["src1w"], dst1w=pr["dst1w"],
            M2=pr["l2"][k]["M2"],
            src2w=pr["l2"][k]["src2w"], dst2w=pr["l2"][k]["dst2w"],
            jlw=pr["jlw"],
            uiw=pr["uiw"][k], biw=pr["biw"][k],
            u_mask_b=pr["u_mask_b"], bm025_b=pr["bm025_b"],
            ident=pr["ident"],
        )
        if pr["has_b1"]:
            m["b1_b"] = np.broadcast_to(
                inp["b1"][k * H:(k + 1) * H].astype(f32), (P, H)).copy()
        if pr["has_b2"]:
            m["b2_b"] = np.broadcast_to(inp["b2"].astype(f32), (P, H)).copy()
        in_maps.append(m)
    return in_maps


def run(inputs, debug=False, want_results=False):
    pr = host_prep(inputs)
    nc = build_program(pr, debug=debug)
    in_maps = make_in_maps(pr)
    res = bass_utils.run_bass_kernel_spmd(
        nc, in_maps, core_ids=list(range(NCORES)), trace=False)
    y = np.concatenate([res.results[k]["y"][:, 0] for k in range(NCORES)])
    if want_results:
        return y.astype(np.float32), res, pr, nc, in_maps
    return y.astype(np.float32)


def kernel(**inputs):
    return run(inputs)

